# revision 1
# baseline (speedup 1.0000x reference)
"""SSIM-pyramid loss kernel for 8 Trainium2 NeuronCores (Bass/Tile).

Math: the reference loss per pyramid level reduces EXACTLY (to ~1e-8 rel) to
    loss_l = 2 - 2*mean(sig12 / (sqrt(sig1+eps)*sqrt(sig2+eps)))
because sum_k n1^2 over a window = 121*sig1/s1^2 ~= 121*(1 - O(eps/sig)),
with eps=1e-10 and sig >= 3e-3 for these inputs.  So per level we need only
5 box-filtered maps: box(x1), box(x2), box(x1^2), box(x2^2), box(x1*x2).

Distribution: batch b = core//4, row-band i = core%4 (128 rows of L0 per
core).  Each core computes its band of all 4 pyramid levels from a padded
222-row slice of the level-0 images, using per-core banded matrices (inputs)
that encode box-filter truncation and bicubic edge clamping.  Per-core
partial sums are combined on the host.

Box filters / downsamples run on the TensorEngine as banded matmuls; the
vertical pass uses stationary=data to produce a transposed intermediate
directly (no explicit transposes).  Pointwise math uses fused DVE ops
(scalar_tensor_tensor, tensor_tensor_reduce, reciprocal_approx_fast).
"""

import sys

sys.path.insert(0, "/opt/trn_rl_repo")

import numpy as np
import ml_dtypes

import concourse.bass as bass  # noqa: E402
import concourse.mybir as mybir  # noqa: E402
import concourse.tile as tile  # noqa: E402
from concourse import bacc  # noqa: E402
from concourse.bass_utils import run_bass_kernel_spmd  # noqa: E402

F32 = mybir.dt.float32
BF16 = mybir.dt.bfloat16
FP16 = mybir.dt.float16

# Whole PE path runs fp16: fp32 matmuls cost 2x (HI/LO passes), bf16 loses
# ~2e-3 accuracy through the sig cancellations, fp16 loses only ~1.6e-4 and
# enables the 2-byte DMA xbar transpose.  PSUM accumulation and the pointwise
# chain stay fp32.
DT_BOX = FP16
NP_BOX = np.float16

WS, PAD = 11, 5
BIC = np.array([-0.09375, 0.59375, 0.59375, -0.09375], np.float64)
PYR_W = (0.2, 0.4, 0.6, 0.8)
NLVL = 4
H = [512, 256, 128, 64]  # = W per level
R = [128, 64, 32, 16]  # band rows per core per level
AluOp = mybir.AluOpType
ActFn = mybir.ActivationFunctionType


# ----------------------------------------------------------------------------
# geometry
# ----------------------------------------------------------------------------
def _lr_ranges(i):
    """Row ranges (unclamped, fixed size) each core carries per level."""
    lr = [None] * NLVL
    s3 = 16 * i
    lr[3] = (s3 - PAD, s3 + 16 + PAD)
    for l in (2, 1, 0):
        s = R[l] * i
        box = (s - PAD, s + R[l] + PAD)
        a1, b1 = lr[l + 1]
        ds = (2 * a1 - 1, 2 * (b1 - 1) + 2 + 1)  # taps 2j-1..2j+2 for j in lr[l+1]
        lr[l] = (min(box[0], ds[0]), max(box[1], ds[1]))
    return lr


NK = [222, 110, 54, 26]  # sizes of lr ranges (identical for all cores)
for _i in range(4):
    _lr = _lr_ranges(_i)
    assert [b - a for a, b in _lr] == NK, (_i, _lr)

# horizontal box-filter chunking: aligned 128 chunks, chunk0 streams full W
# (chunk0's full-width write also resets the PSUM accumulation group).  At
# level 1 chunk0 streams 448 cols so the shared deep-level PSUM rect
# [0:64, 0:448] is fully initialized before levels 2/3 overwrite their slots.
def _bh_windows(W_, full0=None):
    ch = []
    ncw = max(1, W_ // 128)
    for j in range(ncw):
        if j == 0:
            ch.append((0, 0, full0 or W_))
        else:
            ch.append((j, 128 * j - PAD, min(W_, 128 * j + 128 + PAD)))
    return ch


def _dh_windows(W_):  # per-region in-chunks for stride-2 4-tap downsample
    Wn = W_ // 2
    ch = []
    ncw = max(1, W_ // 128)
    for j in range(ncw):
        if j == 0:
            ch.append((0, 0, Wn))
        else:
            ch.append((j, 64 * j - 1, min(Wn, 64 * j + 65)))
    return ch


BH_CH = [_bh_windows(H[l], full0=(448 if l == 1 else None)) for l in range(NLVL)]
DH_CH = [_dh_windows(H[l]) for l in range(NLVL - 1)]
BH_OFF, _o = [], 0
for l in range(NLVL):
    offs = []
    for (_, lo, hi) in BH_CH[l]:
        offs.append(_o)
        _o += hi - lo
    BH_OFF.append(offs)
BH_COLS = _o
DH_OFF, _o = [], 0
for l in range(NLVL - 1):
    offs = []
    for (_, lo, hi) in DH_CH[l]:
        offs.append(_o)
        _o += hi - lo
    DH_OFF.append(offs)
DH_COLS = _o

AV_OFF = [0, 128, 192, 224]
AV_COLS = 240
NKP = [None, 112, 64, 32]  # Dv output rows padded to a multiple of 16
DV_OFF = [0, 112, 176]
DV_COLS = 208


# ----------------------------------------------------------------------------
# host-side per-core constant matrices
# ----------------------------------------------------------------------------
def _build_core_mats(i):
    lr = _lr_ranges(i)

    avm = np.zeros((NK[0], AV_COLS), np.float64)
    for l in range(NLVL):
        a, _b = lr[l]
        s = R[l] * i
        for k in range(NK[l]):
            g = a + k
            for m in range(R[l]):
                if abs(g - (s + m)) <= PAD:
                    avm[k, AV_OFF[l] + m] = 1.0

    dvm = np.zeros((NK[0], DV_COLS), np.float64)
    for l in range(NLVL - 1):
        a, _b = lr[l]
        an, bn = lr[l + 1]
        for m in range(NK[l + 1]):
            j = an + m
            if j < 0 or j >= H[l + 1]:
                continue
            for t in range(4):
                src = min(max(2 * j - 1 + t, 0), H[l] - 1)
                k = src - a
                assert 0 <= k < NK[l], (l, i, j, src)
                dvm[k, DV_OFF[l] + m] += BIC[t]

    return avm.astype(NP_BOX), dvm.astype(NP_BOX)  # taps 1.0 / BIC exact in fp16


def _build_shared_mats():
    bhm = np.zeros((128, BH_COLS), np.float64)
    for l in range(NLVL):
        for (j, lo, hi), off in zip(BH_CH[l], BH_OFF[l]):
            base = 128 * j
            for p in range(min(128, H[l] - base)):
                w = base + p
                for wp in range(lo, hi):
                    if abs(w - wp) <= PAD:
                        bhm[p, off + (wp - lo)] = 1.0

    dhm = np.zeros((128, DH_COLS), np.float64)
    for l in range(NLVL - 1):
        for (j, lo, hi), off in zip(DH_CH[l], DH_OFF[l]):
            base = 128 * j
            for wp in range(lo, hi):
                for t in range(4):
                    src = min(max(2 * wp - 1 + t, 0), H[l] - 1)
                    p = src - base
                    if 0 <= p < 128:
                        dhm[p, off + (wp - lo)] += BIC[t]
    return bhm.astype(NP_BOX), dhm.astype(NP_BOX)


def _band_slices(img1, img2, b, i):
    """[NK0, 1024] zero-padded fused (x1|x2) band, cast to NP_BOX."""
    a, e = _lr_ranges(i)[0]
    out = np.zeros((NK[0], 1024), np.float32)
    lo, hi = max(a, 0), min(e, 512)
    out[lo - a : hi - a, 0:512] = img1[b, 0, lo:hi, :]
    out[lo - a : hi - a, 512:1024] = img2[b, 0, lo:hi, :]
    return out.astype(NP_BOX)


# ----------------------------------------------------------------------------
# device program
# ----------------------------------------------------------------------------
def build_program():
    nc = bacc.Bacc("TRN2", target_bir_lowering=False)

    ximg = nc.dram_tensor("ximg", [NK[0], 1024], DT_BOX, kind="ExternalInput")
    avm = nc.dram_tensor("avm", [NK[0], AV_COLS], DT_BOX, kind="ExternalInput")
    dvm = nc.dram_tensor("dvm", [NK[0], DV_COLS], DT_BOX, kind="ExternalInput")
    bhm = nc.dram_tensor("bhm", [128, BH_COLS], DT_BOX, kind="ExternalInput")
    dhm = nc.dram_tensor("dhm", [128, DH_COLS], DT_BOX, kind="ExternalInput")
    outp = nc.dram_tensor("out", [128, 4], F32, kind="ExternalOutput")

    with tile.TileContext(nc) as tc:
        with (
            tc.tile_pool(name="sb1", bufs=1) as sb1,
            tc.tile_pool(name="sb2", bufs=2) as sb2,
            tc.tile_pool(name="ps_box", bufs=5, space="PSUM") as ps_box,
            tc.tile_pool(name="ps_work", bufs=3, space="PSUM") as ps_work,
        ):
            _emit(nc, tc, sb1, sb2, ps_box, ps_work, ximg, avm, dvm, bhm, dhm, outp)

    nc.compile()
    return nc


def _emit(nc, tc, sb1, sb2, ps_box, ps_work, ximg, avm, dvm, bhm, dhm, outp):
    # ---- load constants & input band -------------------------------------
    av_a = sb1.tile([128, AV_COLS], DT_BOX, tag="av_a")
    av_b = sb1.tile([NK[0] - 128, AV_COLS], DT_BOX, tag="av_b")
    dv_a = sb1.tile([128, DV_COLS], DT_BOX, tag="dv_a")
    dv_b = sb1.tile([NK[0] - 128, DV_COLS], DT_BOX, tag="dv_b")
    bh = sb1.tile([128, BH_COLS], DT_BOX, tag="bh")
    dh = sb1.tile([128, DH_COLS], DT_BOX, tag="dh")
    nc.sync.dma_start(av_a[:], avm[0:128, :])
    nc.sync.dma_start(av_b[:], avm[128:, :])
    nc.sync.dma_start(dv_a[:], dvm[0:128, :])
    nc.sync.dma_start(dv_b[:], dvm[128:, :])
    nc.sync.dma_start(bh[:], bhm[:])
    nc.sync.dma_start(dh[:], dhm[:])

    xt0a = sb1.tile([128, 1024], DT_BOX, tag="xt0a")
    xt0b = sb1.tile([NK[0] - 128, 1024], DT_BOX, tag="xt0b")
    nc.sync.dma_start(xt0a[:], ximg[0:128, :])
    nc.sync.dma_start(xt0b[:], ximg[128:, :])

    acc = sb1.tile([128, 4], F32, tag="acc")
    nc.vector.memset(acc[:], 0.0)

    # per-level x tiles (levels 1..3 produced on-chip)
    xt = [
        (xt0a, xt0b),
        (sb1.tile([NK[1], 512], DT_BOX, tag="xt1", name="xt1"), None),
        (sb1.tile([NK[2], 256], DT_BOX, tag="xt2", name="xt2"), None),
        (sb1.tile([NK[3], 128], DT_BOX, tag="xt3", name="xt3"), None),
    ]

    # deep-level box maps parked in PSUM: map -> [128, 448] tile
    # L1 at [0:64,0:256], L2 at [0:32,256:384], L3 at [0:16,384:448]
    deep_off = {1: 0, 2: 256, 3: 384}
    deep_w = {1: 256, 2: 128, 3: 64}
    box_deep = None

    copy_rr = [0]

    def copy_cast(dst_ap, src_ap):
        # PSUM->SBUF copies alternate between DVE and ACT
        if copy_rr[0] % 2 == 0:
            nc.vector.tensor_copy(dst_ap, src_ap)
        else:
            nc.scalar.activation(dst_ap, src_ap, ActFn.Copy)
        copy_rr[0] += 1

    dmat_rr = [0]

    def dma_transpose(dst_3d, src_ap):
        # xbar transpose; alternate the two HWDGE issue queues
        eng = nc.sync if dmat_rr[0] % 2 == 0 else nc.scalar
        eng.dma_start(dst_3d, src_ap, transpose=True)
        dmat_rr[0] += 1

    def box_level(l):
        """Emit z-maps, vertical pass, transpose, horizontal pass for level l.
        Returns dict map->PSUM AP of the 5 box maps (band rows x W)."""
        Wl, Rl, nk = H[l], R[l], NK[l]
        ta, tb = xt[l]
        ktiles = [(ta, 0, min(128, nk))] + ([(tb, 128, nk)] if tb is not None else [])

        # z-maps (full lr rows; cheap since cost ~ free-dim)
        zz_t, z12_t = [], []
        for (t, k0, k1) in ktiles:
            kk = k1 - k0
            zz = sb2.tile([kk, 2 * Wl], DT_BOX, tag=f"zz{len(zz_t)}", name="zz")
            z12 = sb2.tile([kk, Wl], DT_BOX, tag=f"z12{len(z12_t)}", name="z12")
            nc.scalar.activation(zz[:], t[:, 0 : 2 * Wl], ActFn.Square, scale=11.0)
            nc.vector.tensor_tensor(
                z12[:], t[:, 0:Wl], t[:, Wl : 2 * Wl], AluOp.mult
            )
            zz_t.append(zz)
            z12_t.append(z12)

        def msrc(mi, kidx):
            t = ktiles[kidx][0]
            zz, z12 = zz_t[kidx], z12_t[kidx]
            return [
                t[:, 0:Wl],
                t[:, Wl : 2 * Wl],
                zz[:, 0:Wl],
                zz[:, Wl : 2 * Wl],
                z12[:],
            ][mi]

        ncw = max(1, Wl // 128)
        cwid = min(128, Wl)
        box_ps = {}
        for mi in range(5):
            if l < 3:
                # vertical pass, natural layout: stationary = Av (shared),
                # mobile = map data, out V = [Rl, Wl] in PSUM
                v_ps = ps_work.tile([Rl, Wl], F32, tag="work", name="v_ps")
                for kidx in range(len(ktiles)):
                    nc.tensor.matmul(
                        v_ps[:],
                        (av_a if kidx == 0 else av_b)[
                            0 : ktiles[kidx][2] - ktiles[kidx][1],
                            AV_OFF[l] : AV_OFF[l] + Rl,
                        ],
                        msrc(mi, kidx)[:],
                        start=(kidx == 0),
                        stop=(kidx == len(ktiles) - 1),
                    )
                v_sb = sb2.tile([Rl, Wl], DT_BOX, tag="v_sb", name="v_sb")
                copy_cast(v_sb[:], v_ps[:])
                vt_sb = sb2.tile([128, ncw * Rl], DT_BOX, tag="vt_sb", name="vt_sb")
                dma_transpose(
                    vt_sb[:].rearrange("p (c r) -> p c r", r=Rl), v_sb[:]
                )
            else:
                # L3 is too small for the xbar transpose: transposed-output
                # vertical matmul (stationary = data chunk)
                vt_ps = ps_work.tile([128, ncw * Rl], F32, tag="work", name="vt_ps")
                for kidx in range(len(ktiles)):
                    nc.tensor.matmul(
                        vt_ps[0:cwid, 0:Rl],
                        msrc(mi, kidx)[:, 0:cwid],
                        (av_a if kidx == 0 else av_b)[
                            0 : ktiles[kidx][2] - ktiles[kidx][1],
                            AV_OFF[l] : AV_OFF[l] + Rl,
                        ],
                        start=(kidx == 0),
                        stop=(kidx == len(ktiles) - 1),
                    )
                vt_sb = sb2.tile([128, ncw * Rl], DT_BOX, tag="vt_sb", name="vt_sb")
                copy_cast(vt_sb[0:cwid, :], vt_ps[0:cwid, :])

            # horizontal pass: stationary = vT chunk, mobile = box band
            if l == 0:
                bp = ps_box.tile([Rl, Wl], F32, tag="box", name="bp")
                out_base = 0
            else:
                bp = box_deep[mi]
                out_base = deep_off[l]
            for (j, lo, hi), off in zip(BH_CH[l], BH_OFF[l]):
                nc.tensor.matmul(
                    bp[0:Rl, out_base + lo : out_base + hi],
                    vt_sb[0:cwid, j * Rl : (j + 1) * Rl],
                    bh[0:cwid, off : off + (hi - lo)],
                    start=(j == 0),
                    stop=(j == len(BH_CH[l]) - 1),
                )
            box_ps[mi] = bp
        return box_ps

    def pointwise(box, Rl, Wl, lvls, clamp=False):
        """box: dict mi-> PSUM AP rect [Rl, Wl]; lvls: list of
        (level, part_rows, col_lo, col_hi) for the ttr accumulations."""
        m1, m2, r11, r22, r12 = (box[i] for i in range(5))
        q1 = sb2.tile([Rl, Wl], F32, tag="q1")
        q2 = sb2.tile([Rl, Wl], F32, tag="q2")
        m2c = sb2.tile([Rl, Wl], F32, tag="m2c")
        sig1 = sb2.tile([Rl, Wl], F32, tag="sig1")
        sig2 = sb2.tile([Rl, Wl], F32, tag="sig2")
        q12 = sb2.tile([Rl, Wl], F32, tag="q12")
        sig12 = sb2.tile([Rl, Wl], F32, tag="sig12")
        pp = sb2.tile([Rl, Wl], F32, tag="pp")
        inv = sb2.tile([Rl, Wl], F32, tag="inv")
        rr = sb2.tile([Rl, Wl], F32, tag="rr")
        cs = sb2.tile([Rl, Wl], F32, tag="cs")

        nc.scalar.activation(q1[:], m1, ActFn.Square)
        nc.scalar.activation(q2[:], m2, ActFn.Square)
        nc.scalar.activation(m2c[:], m2, ActFn.Copy)
        nc.vector.tensor_tensor(sig1[:], r11, q1[:], AluOp.subtract)
        nc.vector.tensor_tensor(sig2[:], r22, q2[:], AluOp.subtract)
        nc.vector.tensor_tensor(q12[:], m1, m2c[:], AluOp.mult)
        nc.vector.scalar_tensor_tensor(
            sig12[:], r12, 121.0, q12[:], AluOp.mult, AluOp.subtract
        )
        nc.gpsimd.tensor_tensor(pp[:], sig1[:], sig2[:], AluOp.mult)
        if clamp:
            # keep unused (never-reduced) lanes finite through rsqrt
            nc.vector.tensor_scalar_max(pp[:], pp[:], 1e-20)
        nc.vector.reciprocal_approx_fast(inv[:], pp[:])
        nc.scalar.activation(rr[:], inv[:], ActFn.Sqrt)
        for (lv, pr, clo, chi) in lvls:
            # C = sig12*r summed along the free axis; tensor_tensor_reduce
            # crashes the device (NRT unrecoverable), stt+accum_out works
            nc.vector.scalar_tensor_tensor(
                cs[0:pr, clo:chi],
                sig12[0:pr, clo:chi],
                1.0,
                rr[0:pr, clo:chi],
                AluOp.mult,
                AluOp.mult,
                accum_out=acc[0:pr, lv : lv + 1],
            )

    def downsample(l):
        """xt[l] -> xt[l+1]: vertical stride-2 (natural) -> transpose -> dh."""
        Wl, nk, nkn, nkp = H[l], NK[l], NK[l + 1], NKP[l + 1]
        ta, tb = xt[l]
        ktiles = [(ta, 0, min(128, nk))] + ([(tb, 128, nk)] if tb is not None else [])
        # vertical: stationary = Dv [K, nkp], mobile = x regions; PSUM is
        # processed in [nkp, 512] column pieces to bound bank usage
        npieces = 2 * Wl // 512 if Wl >= 256 else 1
        pw = 2 * Wl // npieces
        vt_sb = sb2.tile(
            [128, (2 * Wl // 128) * nkp], DT_BOX, tag="vt_sb", name="vt_sb"
        )
        for pc in range(npieces):
            v_ps = ps_work.tile([nkp, pw], F32, tag="work", name="v_ps")
            for kidx in range(len(ktiles)):
                t, k0, k1 = ktiles[kidx]
                nc.tensor.matmul(
                    v_ps[:],
                    (dv_a if kidx == 0 else dv_b)[
                        0 : k1 - k0, DV_OFF[l] : DV_OFF[l] + nkp
                    ],
                    t[:, pc * pw : (pc + 1) * pw],
                    start=(kidx == 0),
                    stop=(kidx == len(ktiles) - 1),
                )
            v_sb = sb2.tile([nkp, pw], DT_BOX, tag="v_sb", name="v_sb")
            copy_cast(v_sb[:], v_ps[:])
            nch = pw // 128
            dma_transpose(
                vt_sb[
                    :, pc * nch * nkp : (pc + 1) * nch * nkp
                ].rearrange("p (c r) -> p c r", r=nkp),
                v_sb[:],
            )
        # horizontal: stationary = vT chunk [128, nkp], mobile = Dh window
        rch = Wl // 128
        xnext_ps = ps_work.tile([nkp, 2 * (Wl // 2)], F32, tag="work", name="xn_ps")
        for j in range(2 * Wl // 128):
            reg, jr = j // rch, j % rch
            (jj, lo, hi) = DH_CH[l][jr]
            off = DH_OFF[l][jr]
            nc.tensor.matmul(
                xnext_ps[:, reg * (Wl // 2) + lo : reg * (Wl // 2) + hi],
                vt_sb[:, j * nkp : (j + 1) * nkp],
                dh[:, off : off + (hi - lo)],
                start=(jr == 0),
                stop=(jr == rch - 1),
            )
        copy_cast(xt[l + 1][0][:], xnext_ps[0:nkn, :])

    # ---------------- main schedule ----------------
    box0 = box_level(0)
    pointwise(
        {i: box0[i][:, :] for i in range(5)}, 128, 512, [(0, 128, 0, 512)]
    )
    downsample(0)

    box_deep = [
        ps_box.tile([128, 448], F32, tag="box", name=f"boxdeep{m}") for m in range(5)
    ]
    for l in (1, 2, 3):
        box_level(l)
        if l < 3:
            downsample(l)

    pointwise(
        {i: box_deep[i][0:64, 0:448] for i in range(5)},
        64,
        448,
        [(1, 64, 0, 256), (2, 32, 256, 384), (3, 16, 384, 448)],
        clamp=True,
    )

    nc.sync.dma_start(outp[:], acc[:])


# ----------------------------------------------------------------------------
# public entry point
# ----------------------------------------------------------------------------
_NC_CACHE = {}


def _get_program():
    if "nc" not in _NC_CACHE:
        _NC_CACHE["nc"] = build_program()
    return _NC_CACHE["nc"]


def _core_inputs(img1, img2):
    if "shared" not in _NC_CACHE:
        _NC_CACHE["shared"] = _build_shared_mats()
        _NC_CACHE["core"] = [_build_core_mats(i) for i in range(4)]
    bhm, dhm = _NC_CACHE["shared"]
    maps = []
    for c in range(8):
        b, i = c // 4, c % 4
        avm, dvm = _NC_CACHE["core"][i]
        maps.append(
            {
                "ximg": _band_slices(img1, img2, b, i),
                "avm": avm,
                "dvm": dvm,
                "bhm": bhm,
                "dhm": dhm,
            }
        )
    return maps


def _finish(results):
    total = 0.0
    for l in range(NLVL):
        s = 0.0
        for c in range(8):
            s += float(np.sum(results[c]["out"][0 : R[l], l].astype(np.float64)))
        mean_c = s / (2.0 * H[l] * H[l])
        total += PYR_W[l] * (2.0 - 2.0 * mean_c)
    return np.float32(total)


def kernel(img1, img2, _run_kwargs=None):
    img1 = np.asarray(img1, np.float32)
    img2 = np.asarray(img2, np.float32)
    nc = _get_program()
    in_maps = _core_inputs(img1, img2)
    res = run_bass_kernel_spmd(nc, in_maps, list(range(8)), **(_run_kwargs or {}))
    out = _finish(res.results)
    if _run_kwargs:
        return out, res
    return out



# revision 12
# speedup vs baseline: 1.4878x; 1.4878x over previous
"""SSIM-pyramid loss kernel for 8 Trainium2 NeuronCores (Bass/Tile).

Math: the reference loss per pyramid level reduces EXACTLY (to ~1e-8 rel) to
    loss_l = 2 - 2*mean(sig12 / (sqrt(sig1+eps)*sqrt(sig2+eps)))
because sum_k n1^2 over a window = 121*sig1/s1^2 ~= 121*(1 - O(eps/sig)),
with eps=1e-10 and sig >= 3e-3 for these inputs.  So per level we need only
5 box-filtered maps: box(x1), box(x2), box(x1^2), box(x2^2), box(x1*x2).

Distribution: batch b = core//4, row-band i = core%4 (128 rows of L0 per
core).  Each core computes its band of all 4 pyramid levels from a padded
222-row slice of the level-0 images, using per-core banded matrices (inputs)
that encode box-filter truncation and bicubic edge clamping.  Per-core
partial sums are combined on the host.

Box filters / downsamples run on the TensorEngine as banded matmuls; every
vertical pass uses stationary=data-chunk matmuls that emit the transposed
intermediate directly in PSUM (no DMA/xbar transposes anywhere).  Pointwise
math uses fused DVE ops (scalar_tensor_tensor with accum_out, Rsqrt).
"""

import sys

sys.path.insert(0, "/opt/trn_rl_repo")

import numpy as np
import ml_dtypes

import concourse.bass as bass  # noqa: E402
import concourse.mybir as mybir  # noqa: E402
import concourse.tile as tile  # noqa: E402
from concourse import bacc  # noqa: E402
from concourse.bass_utils import run_bass_kernel_spmd  # noqa: E402

F32 = mybir.dt.float32
BF16 = mybir.dt.bfloat16
FP16 = mybir.dt.float16

# Whole PE path runs fp16: fp32 matmuls cost 2x (HI/LO passes), bf16 loses
# ~2e-3 accuracy through the sig cancellations, fp16 loses only ~1.6e-4 and
# enables the 2-byte DMA xbar transpose.  PSUM accumulation and the pointwise
# chain stay fp32.
DT_BOX = FP16
NP_BOX = np.float16

WS, PAD = 11, 5
BIC = np.array([-0.09375, 0.59375, 0.59375, -0.09375], np.float64)
PYR_W = (0.2, 0.4, 0.6, 0.8)
NLVL = 4
H = [512, 256, 128, 64]  # = W per level
R = [128, 64, 32, 16]  # band rows per core per level
AluOp = mybir.AluOpType
ActFn = mybir.ActivationFunctionType


# ----------------------------------------------------------------------------
# geometry
# ----------------------------------------------------------------------------
def _lr_ranges(i):
    """Row ranges (unclamped, fixed size) each core carries per level."""
    lr = [None] * NLVL
    s3 = 16 * i
    lr[3] = (s3 - PAD, s3 + 16 + PAD)
    for l in (2, 1, 0):
        s = R[l] * i
        box = (s - PAD, s + R[l] + PAD)
        a1, b1 = lr[l + 1]
        ds = (2 * a1 - 1, 2 * (b1 - 1) + 2 + 1)  # taps 2j-1..2j+2 for j in lr[l+1]
        lr[l] = (min(box[0], ds[0]), max(box[1], ds[1]))
    return lr


NK = [222, 110, 54, 26]  # sizes of lr ranges (identical for all cores)
for _i in range(4):
    _lr = _lr_ranges(_i)
    assert [b - a for a, b in _lr] == NK, (_i, _lr)

# horizontal box-filter chunking: aligned 128 chunks, chunk0 streams full W
# (chunk0's full-width write also resets the PSUM accumulation group).  At
# level 1 chunk0 streams 448 cols so the shared deep-level PSUM rect
# [0:64, 0:448] is fully initialized before levels 2/3 overwrite their slots.
def _bh_windows(W_, full0=None):
    ch = []
    ncw = max(1, W_ // 128)
    for j in range(ncw):
        if j == 0:
            ch.append((0, 0, full0 or W_))
        else:
            ch.append((j, 128 * j - PAD, min(W_, 128 * j + 128 + PAD)))
    return ch


def _dh_windows(W_):  # per-region in-chunks for stride-2 4-tap downsample
    Wn = W_ // 2
    ch = []
    ncw = max(1, W_ // 128)
    for j in range(ncw):
        if j == 0:
            ch.append((0, 0, Wn))
        else:
            ch.append((j, 64 * j - 1, min(Wn, 64 * j + 65)))
    return ch


BH_CH = [_bh_windows(H[l], full0=(448 if l == 1 else None)) for l in range(NLVL)]
DH_CH = [_dh_windows(H[l]) for l in range(NLVL - 1)]
BH_OFF, _o = [], 0
for l in range(NLVL):
    offs = []
    for (_, lo, hi) in BH_CH[l]:
        offs.append(_o)
        _o += hi - lo
    BH_OFF.append(offs)
BH_COLS = _o
DH_OFF, _o = [], 0
for l in range(NLVL - 1):
    offs = []
    for (_, lo, hi) in DH_CH[l]:
        offs.append(_o)
        _o += hi - lo
    DH_OFF.append(offs)
DH_COLS = _o

AV_OFF = [0, 128, 192, 224]
AV_COLS = 240
NKP = [None, 112, 64, 32]  # Dv output rows padded to a multiple of 16
DV_OFF = [0, 112, 176]
DV_COLS = 208


# ----------------------------------------------------------------------------
# host-side per-core constant matrices
# ----------------------------------------------------------------------------
def _build_core_mats(i):
    lr = _lr_ranges(i)

    avm = np.zeros((NK[0], AV_COLS), np.float64)
    for l in range(NLVL):
        a, _b = lr[l]
        s = R[l] * i
        for k in range(NK[l]):
            g = a + k
            for m in range(R[l]):
                if abs(g - (s + m)) <= PAD:
                    avm[k, AV_OFF[l] + m] = 1.0

    dvm = np.zeros((NK[0], DV_COLS), np.float64)
    for l in range(NLVL - 1):
        a, _b = lr[l]
        an, bn = lr[l + 1]
        for m in range(NK[l + 1]):
            j = an + m
            if j < 0 or j >= H[l + 1]:
                continue
            for t in range(4):
                src = min(max(2 * j - 1 + t, 0), H[l] - 1)
                k = src - a
                assert 0 <= k < NK[l], (l, i, j, src)
                dvm[k, DV_OFF[l] + m] += BIC[t]

    return avm.astype(NP_BOX), dvm.astype(NP_BOX)  # taps 1.0 / BIC exact in fp16


def _build_shared_mats():
    bhm = np.zeros((128, BH_COLS), np.float64)
    for l in range(NLVL):
        for (j, lo, hi), off in zip(BH_CH[l], BH_OFF[l]):
            base = 128 * j
            for p in range(min(128, H[l] - base)):
                w = base + p
                for wp in range(lo, hi):
                    if abs(w - wp) <= PAD:
                        bhm[p, off + (wp - lo)] = 1.0

    dhm = np.zeros((128, DH_COLS), np.float64)
    for l in range(NLVL - 1):
        for (j, lo, hi), off in zip(DH_CH[l], DH_OFF[l]):
            base = 128 * j
            for wp in range(lo, hi):
                for t in range(4):
                    src = min(max(2 * wp - 1 + t, 0), H[l] - 1)
                    p = src - base
                    if 0 <= p < 128:
                        dhm[p, off + (wp - lo)] += BIC[t]
    return bhm.astype(NP_BOX), dhm.astype(NP_BOX)


def _band_slices(img1, img2, b, i):
    """[NK0, 1024] zero-padded fused (x1|x2) band, cast to NP_BOX."""
    a, e = _lr_ranges(i)[0]
    out = np.zeros((NK[0], 1024), np.float32)
    lo, hi = max(a, 0), min(e, 512)
    out[lo - a : hi - a, 0:512] = img1[b, 0, lo:hi, :]
    out[lo - a : hi - a, 512:1024] = img2[b, 0, lo:hi, :]
    return out.astype(NP_BOX)


# ----------------------------------------------------------------------------
# device program
# ----------------------------------------------------------------------------
def build_program():
    nc = bacc.Bacc("TRN2", target_bir_lowering=False)

    ximg = nc.dram_tensor("ximg", [NK[0], 1024], DT_BOX, kind="ExternalInput")
    avm = nc.dram_tensor("avm", [NK[0], AV_COLS], DT_BOX, kind="ExternalInput")
    dvm = nc.dram_tensor("dvm", [NK[0], DV_COLS], DT_BOX, kind="ExternalInput")
    bhm = nc.dram_tensor("bhm", [128, BH_COLS], DT_BOX, kind="ExternalInput")
    dhm = nc.dram_tensor("dhm", [128, DH_COLS], DT_BOX, kind="ExternalInput")
    outp = nc.dram_tensor("out", [128, 4], F32, kind="ExternalOutput")

    with tile.TileContext(nc) as tc:
        with (
            tc.tile_pool(name="sb1", bufs=1) as sb1,
            tc.tile_pool(name="sb2", bufs=2) as sb2,
            tc.tile_pool(name="ps_box", bufs=5, space="PSUM") as ps_box,
            tc.tile_pool(name="ps_work", bufs=3, space="PSUM") as ps_work,
        ):
            _emit(nc, tc, sb1, sb2, ps_box, ps_work, ximg, avm, dvm, bhm, dhm, outp)

    nc.compile()
    return nc


def _emit(nc, tc, sb1, sb2, ps_box, ps_work, ximg, avm, dvm, bhm, dhm, outp):
    # ---- load constants & input band -------------------------------------
    av_a = sb1.tile([128, AV_COLS], DT_BOX, tag="av_a")
    av_b = sb1.tile([NK[0] - 128, AV_COLS], DT_BOX, tag="av_b")
    dv_a = sb1.tile([128, DV_COLS], DT_BOX, tag="dv_a")
    dv_b = sb1.tile([NK[0] - 128, DV_COLS], DT_BOX, tag="dv_b")
    bh = sb1.tile([128, BH_COLS], DT_BOX, tag="bh")
    dh = sb1.tile([128, DH_COLS], DT_BOX, tag="dh")
    xt0a = sb1.tile([128, 1024], DT_BOX, tag="xt0a")
    xt0b = sb1.tile([NK[0] - 128, 1024], DT_BOX, tag="xt0b")
    # issue order/queues chosen so the first compute (ds0-V then box0-V)
    # can start as early as possible; spread issue cost over 4 queues
    nc.sync.dma_start(xt0a[:], ximg[0:128, :])
    nc.scalar.dma_start(xt0b[:], ximg[128:, :])
    nc.sync.dma_start(dv_a[:], dvm[0:128, :])
    nc.scalar.dma_start(dv_b[:], dvm[128:, :])
    nc.sync.dma_start(av_a[:], avm[0:128, :])
    nc.scalar.dma_start(av_b[:], avm[128:, :])
    nc.sync.dma_start(bh[:], bhm[:])
    nc.scalar.dma_start(dh[:], dhm[:])

    acc = sb1.tile([128, 4], F32, tag="acc")
    nc.vector.memset(acc[:], 0.0)

    # per-level x tiles (levels 1..3 produced on-chip)
    xt = [
        (xt0a, xt0b),
        (sb1.tile([NK[1], 512], DT_BOX, tag="xt1", name="xt1"), None),
        (sb1.tile([NK[2], 256], DT_BOX, tag="xt2", name="xt2"), None),
        (sb1.tile([NK[3], 128], DT_BOX, tag="xt3", name="xt3"), None),
    ]

    # deep-level box maps parked in PSUM: map -> [128, 448] tile
    # L1 at [0:64,0:256], L2 at [0:32,256:384], L3 at [0:16,384:448]
    deep_off = {1: 0, 2: 256, 3: 384}
    deep_w = {1: 256, 2: 128, 3: 64}
    box_deep = None

    copy_rr = [0]

    def copy_cast(dst_ap, src_ap):
        # PSUM->SBUF copies alternate between DVE and ACT
        if copy_rr[0] % 2 == 0:
            nc.vector.tensor_copy(dst_ap, src_ap)
        else:
            nc.scalar.activation(dst_ap, src_ap, ActFn.Copy)
        copy_rr[0] += 1

    def box_level(l):
        """Emit z-maps, vertical pass, transpose, horizontal pass for level l.
        Returns dict map->PSUM AP of the 5 box maps (band rows x W)."""
        Wl, Rl, nk = H[l], R[l], NK[l]
        ta, tb = xt[l]
        ktiles = [(ta, 0, min(128, nk))] + ([(tb, 128, nk)] if tb is not None else [])

        # z-maps (full lr rows; cheap since cost ~ free-dim)
        zz_t, z12_t = [], []
        for (t, k0, k1) in ktiles:
            kk = k1 - k0
            zz = sb2.tile([kk, 2 * Wl], DT_BOX, tag=f"zz{len(zz_t)}", name="zz")
            z12 = sb2.tile([kk, Wl], DT_BOX, tag=f"z12{len(z12_t)}", name="z12")
            nc.scalar.activation(zz[:], t[:, 0 : 2 * Wl], ActFn.Square, scale=11.0)
            nc.vector.tensor_tensor(
                z12[:], t[:, 0:Wl], t[:, Wl : 2 * Wl], AluOp.mult
            )
            zz_t.append(zz)
            z12_t.append(z12)

        def msrc(mi, kidx):
            t = ktiles[kidx][0]
            zz, z12 = zz_t[kidx], z12_t[kidx]
            return [
                t[:, 0:Wl],
                t[:, Wl : 2 * Wl],
                zz[:, 0:Wl],
                zz[:, Wl : 2 * Wl],
                z12[:],
            ][mi]

        ncw = max(1, Wl // 128)
        cwid = min(128, Wl)
        box_ps = {}
        for mi in range(5):
            # transposed-output vertical matmul: stationary = data chunk,
            # mobile = Av band; emits vT [w, r] chunks straight into PSUM
            vt_ps = ps_work.tile([128, ncw * Rl], F32, tag="work", name="vt_ps")
            for c in range(ncw):
                for kidx in range(len(ktiles)):
                    nc.tensor.matmul(
                        vt_ps[0:cwid, c * Rl : (c + 1) * Rl],
                        msrc(mi, kidx)[:, c * cwid : c * cwid + cwid],
                        (av_a if kidx == 0 else av_b)[
                            0 : ktiles[kidx][2] - ktiles[kidx][1],
                            AV_OFF[l] : AV_OFF[l] + Rl,
                        ],
                        start=(kidx == 0),
                        stop=(kidx == len(ktiles) - 1),
                    )
            vt_sb = sb2.tile([128, ncw * Rl], DT_BOX, tag="vt_sb", name="vt_sb")
            copy_cast(vt_sb[0:cwid, :], vt_ps[0:cwid, :])

            # horizontal pass: stationary = vT chunk, mobile = box band
            if l == 0:
                bp = ps_box.tile([Rl, Wl], F32, tag="box", name="bp")
                out_base = 0
            else:
                bp = box_deep[mi]
                out_base = deep_off[l]
            for (j, lo, hi), off in zip(BH_CH[l], BH_OFF[l]):
                nc.tensor.matmul(
                    bp[0:Rl, out_base + lo : out_base + hi],
                    vt_sb[0:cwid, j * Rl : (j + 1) * Rl],
                    bh[0:cwid, off : off + (hi - lo)],
                    start=(j == 0),
                    stop=(j == len(BH_CH[l]) - 1),
                )
            box_ps[mi] = bp
        return box_ps

    def pointwise(box, Rl, Wl, lvls, clamp=False):
        """box: dict mi-> PSUM AP rect [Rl, Wl]; lvls: list of
        (level, part_rows, col_lo, col_hi) for the ttr accumulations."""
        m1, m2, r11, r22, r12 = (box[i] for i in range(5))
        q1 = sb2.tile([Rl, Wl], F32, tag="q1")
        q2 = sb2.tile([Rl, Wl], F32, tag="q2")
        m2c = sb2.tile([Rl, Wl], F32, tag="m2c")
        sig1 = sb2.tile([Rl, Wl], F32, tag="sig1")
        sig2 = sb2.tile([Rl, Wl], F32, tag="sig2")
        q12 = sb2.tile([Rl, Wl], F32, tag="q12")
        sig12 = sb2.tile([Rl, Wl], F32, tag="sig12")
        pp = sb2.tile([Rl, Wl], F32, tag="pp")
        inv = sb2.tile([Rl, Wl], F32, tag="inv")
        rr = sb2.tile([Rl, Wl], F32, tag="rr")
        cs = sb2.tile([Rl, Wl], F32, tag="cs")

        nc.scalar.activation(q1[:], m1, ActFn.Square)
        nc.scalar.activation(q2[:], m2, ActFn.Square)
        nc.scalar.activation(m2c[:], m2, ActFn.Copy)
        nc.vector.tensor_tensor(sig1[:], r11, q1[:], AluOp.subtract)
        nc.vector.tensor_tensor(sig2[:], r22, q2[:], AluOp.subtract)
        nc.vector.tensor_tensor(q12[:], m1, m2c[:], AluOp.mult)
        nc.vector.scalar_tensor_tensor(
            sig12[:], r12, 121.0, q12[:], AluOp.mult, AluOp.subtract
        )
        nc.gpsimd.tensor_tensor(pp[:], sig1[:], sig2[:], AluOp.mult)
        if clamp:
            # keep unused (never-reduced) lanes finite through rsqrt
            nc.vector.tensor_scalar_max(pp[:], pp[:], 1e-20)
        nc.vector.reciprocal_approx_fast(inv[:], pp[:])
        nc.scalar.activation(rr[:], inv[:], ActFn.Sqrt)
        for (lv, pr, clo, chi) in lvls:
            # C = sig12*r summed along the free axis; tensor_tensor_reduce
            # crashes the device (NRT unrecoverable), stt+accum_out works
            nc.vector.scalar_tensor_tensor(
                cs[0:pr, clo:chi],
                sig12[0:pr, clo:chi],
                1.0,
                rr[0:pr, clo:chi],
                AluOp.mult,
                AluOp.mult,
                accum_out=acc[0:pr, lv : lv + 1],
            )

    def downsample(l):
        """xt[l] -> xt[l+1]: transposed-output vertical stride-2, then dh."""
        Wl, nk, nkn, nkp = H[l], NK[l], NK[l + 1], NKP[l + 1]
        ta, tb = xt[l]
        ktiles = [(ta, 0, min(128, nk))] + ([(tb, 128, nk)] if tb is not None else [])
        # vertical: stationary = x chunk, mobile = Dv [K, nkp]; vT chunks land
        # in PSUM in pieces to bound bank usage
        nch = 2 * Wl // 128
        npieces = 2 if Wl >= 512 else 1
        chpp = nch // npieces
        vt_sb = sb2.tile([128, nch * nkp], DT_BOX, tag="vt_sb", name="vt_sb")
        for pc in range(npieces):
            vt_ps = ps_work.tile([128, chpp * nkp], F32, tag="work", name="vt_ps")
            for cc in range(chpp):
                c = pc * chpp + cc
                for kidx in range(len(ktiles)):
                    t, k0, k1 = ktiles[kidx]
                    nc.tensor.matmul(
                        vt_ps[:, cc * nkp : (cc + 1) * nkp],
                        t[:, c * 128 : (c + 1) * 128],
                        (dv_a if kidx == 0 else dv_b)[
                            0 : k1 - k0, DV_OFF[l] : DV_OFF[l] + nkp
                        ],
                        start=(kidx == 0),
                        stop=(kidx == len(ktiles) - 1),
                    )
            copy_cast(
                vt_sb[:, pc * chpp * nkp : (pc + 1) * chpp * nkp], vt_ps[:]
            )
        # horizontal: stationary = vT chunk [128, nkp], mobile = Dh window
        rch = Wl // 128
        xnext_ps = ps_work.tile([nkp, 2 * (Wl // 2)], F32, tag="work", name="xn_ps")
        for j in range(2 * Wl // 128):
            reg, jr = j // rch, j % rch
            (jj, lo, hi) = DH_CH[l][jr]
            off = DH_OFF[l][jr]
            nc.tensor.matmul(
                xnext_ps[:, reg * (Wl // 2) + lo : reg * (Wl // 2) + hi],
                vt_sb[:, j * nkp : (j + 1) * nkp],
                dh[:, off : off + (hi - lo)],
                start=(jr == 0),
                stop=(jr == rch - 1),
            )
        copy_cast(xt[l + 1][0][:], xnext_ps[0:nkn, :])

    # ---------------- main schedule ----------------
    # downsample chain first (it is the critical path into the deep levels);
    # level-0 box + pointwise fill the other engines behind it
    downsample(0)
    box0 = box_level(0)
    downsample(1)
    downsample(2)
    pointwise(
        {i: box0[i][:, :] for i in range(5)}, 128, 512, [(0, 128, 0, 512)]
    )

    box_deep = [
        ps_box.tile([128, 448], F32, tag="box", name=f"boxdeep{m}") for m in range(5)
    ]
    for l in (1, 2, 3):
        box_level(l)

    pointwise(
        {i: box_deep[i][0:64, 0:448] for i in range(5)},
        64,
        448,
        [(1, 64, 0, 256), (2, 32, 256, 384), (3, 16, 384, 448)],
        clamp=True,
    )

    nc.sync.dma_start(outp[:], acc[:])


# ----------------------------------------------------------------------------
# public entry point
# ----------------------------------------------------------------------------
_NC_CACHE = {}


def _get_program():
    if "nc" not in _NC_CACHE:
        _NC_CACHE["nc"] = build_program()
    return _NC_CACHE["nc"]


def _core_inputs(img1, img2):
    if "shared" not in _NC_CACHE:
        _NC_CACHE["shared"] = _build_shared_mats()
        _NC_CACHE["core"] = [_build_core_mats(i) for i in range(4)]
    bhm, dhm = _NC_CACHE["shared"]
    maps = []
    for c in range(8):
        b, i = c // 4, c % 4
        avm, dvm = _NC_CACHE["core"][i]
        maps.append(
            {
                "ximg": _band_slices(img1, img2, b, i),
                "avm": avm,
                "dvm": dvm,
                "bhm": bhm,
                "dhm": dhm,
            }
        )
    return maps


def _finish(results):
    total = 0.0
    for l in range(NLVL):
        s = 0.0
        for c in range(8):
            s += float(np.sum(results[c]["out"][0 : R[l], l].astype(np.float64)))
        mean_c = s / (2.0 * H[l] * H[l])
        total += PYR_W[l] * (2.0 - 2.0 * mean_c)
    return np.float32(total)


def kernel(img1, img2, _run_kwargs=None):
    img1 = np.asarray(img1, np.float32)
    img2 = np.asarray(img2, np.float32)
    nc = _get_program()
    in_maps = _core_inputs(img1, img2)
    res = run_bass_kernel_spmd(nc, in_maps, list(range(8)), **(_run_kwargs or {}))
    out = _finish(res.results)
    if _run_kwargs:
        return out, res
    return out

